# revision 1
# baseline (speedup 1.0000x reference)
"""Multi-head self-attention (16 heads, hd=64, RoPE, causal) on 8 trn2 cores.

Sharding: DP(batch=2) x TP(head-groups=4). Core c handles batch c//4, heads
[4*(c%4), 4*(c%4)+4). Each core computes a row-parallel partial output
yT_partial [1024, 2048]; host sums the 4 partials per batch and transposes.
No device-device communication.

Device kernel works in transposed layout throughout:
  - xT [e, t] streamed from DRAM
  - qT/kT [d_local, t] with per-head de-interleaved RoPE halves (weight rows
    pre-permuted on host so rot1/rot2 operate on contiguous 32-row blocks)
  - scoresT [kt, qt] per head; K=64 matmuls pair-packed via tile_position
  - probs = exp(scoresT) (no max subtraction; scores are O(1) by construction)
  - attnT [vd, qt] = v_aug.T @ probs with M=65 (65th column of v_aug is ones,
    yielding the softmax denominator row for free)
  - yT [e, qt] = woT.T @ attnT_normalized
"""

import sys

for _p in ("/opt/trn_rl_repo",):
    if _p not in sys.path:
        sys.path.insert(0, _p)

import numpy as np

import concourse.bass as bass
import concourse.mybir as mybir
import concourse.tile as tile
from concourse import bacc
from concourse.bass_utils import run_bass_kernel_spmd

F32 = mybir.dt.float32
F32R = mybir.dt.float32r
BF16 = mybir.dt.bfloat16
AF = mybir.ActivationFunctionType

# matmul groups run in float32r (1 cyc/row at N>=256 vs 4 for fp32).
FP32R_GROUPS = {"qkv", "scores", "attnv", "yt", "bcast"}


def _mm(group, ap):
    return ap.bitcast(F32R) if group in FP32R_GROUPS else ap

B, T, E = 2, 2048, 1024
NH, HD = 16, 64
NHL = 4          # heads per core
DL = NHL * HD    # 256 local head dims
NCORES = 8
NEG = -1e9
ROPE_BASE = 10000.0


# ----------------------------------------------------------------- device IR
def build_module(reps=1):
    nc = bacc.Bacc("TRN2", target_bir_lowering=False, debug=False,
                   num_devices=NCORES)

    xt = nc.dram_tensor("xt", [E, T], F32R, kind="ExternalInput").ap()
    wqt = nc.dram_tensor("wqt", [E, DL], F32R, kind="ExternalInput").ap()
    wkt = nc.dram_tensor("wkt", [E, DL], F32R, kind="ExternalInput").ap()
    wvt = nc.dram_tensor("wvt", [E, DL], F32R, kind="ExternalInput").ap()
    wot = nc.dram_tensor("wot", [DL, E], F32R, kind="ExternalInput").ap()
    cd = nc.dram_tensor("cd", [128, T], F32R, kind="ExternalInput").ap()
    sd = nc.dram_tensor("sd", [128, T], F32R, kind="ExternalInput").ap()
    negid = nc.dram_tensor("negid", [128, 128], BF16, kind="ExternalInput").ap()
    stepd = nc.dram_tensor("stepd", [128, 128], BF16, kind="ExternalInput").ap()
    onesd = nc.dram_tensor("onesd", [128, 64], F32R, kind="ExternalInput").ap()
    yt = nc.dram_tensor("yt", [E, T], F32, kind="ExternalOutput").ap()

    with tile.TileContext(nc) as tc:
        for _ in range(reps):
            _body(tc, xt, wqt, wkt, wvt, wot, cd, sd, negid, stepd, onesd, yt)
    nc.compile()
    return nc


def _chunks(qs_rel):
    """512-bank-aligned column chunks of [qs_rel, 1024)."""
    out = []
    if qs_rel < 512:
        out.append((qs_rel, 512))
        out.append((512, 1024))
    else:
        out.append((qs_rel, 1024))
    return out


def _body(tc, xt, wqt, wkt, wvt, wot, cd, sd, negid, stepd, onesd, yt):
    nc = tc.nc
    from contextlib import ExitStack

    with ExitStack() as outer:
        # all SBUF pools at one level: no pool-close gates between phases
        po = outer.enter_context(tc.tile_pool(name="persist", bufs=1))
        wp = outer.enter_context(tc.tile_pool(name="wp", bufs=1))
        xcp = outer.enter_context(tc.tile_pool(name="xcp", bufs=10))
        swpp = outer.enter_context(tc.tile_pool(name="swp", bufs=2))
        expp = outer.enter_context(tc.tile_pool(name="expp", bufs=4))
        dnp = outer.enter_context(tc.tile_pool(name="dnp", bufs=2))
        yp = outer.enter_context(tc.tile_pool(name="yp", bufs=4))

        # persistent tiles
        qk = {}
        for nm in ("q0", "q1", "k0", "k1"):
            qk[nm] = po.tile([128, T], F32R, tag=nm, name=nm)
        v_sb = po.tile([128, 16 * 260], F32R, tag="v")
        wot_sb = [po.tile([128, E], F32R, tag=f"wot{p}", name=f"wot{p}")
                  for p in range(2)]
        at = [po.tile([128, T], F32R, tag=f"at{p}", name=f"at{p}")
              for p in range(2)]
        negi_sb = po.tile([128, 128], BF16, tag="negi")
        step_sb = po.tile([128, 128], BF16, tag="step")
        ones_sb = po.tile([65, 64], F32R, tag="ones")

        w_sb = {}
        w_srcs = {"wq": wqt, "wk": wkt, "wv": wvt}
        for nm in ("wq", "wk", "wv"):
            w_sb[nm] = wp.tile([128, 2048], F32R, tag=nm, name=nm)

        def _wload(nm):
            nc.sync.dma_start(
                out=w_sb[nm][:].rearrange("p (eo d) -> p eo d", eo=8),
                in_=w_srcs[nm].rearrange("(eo p) d -> p eo d", p=128))

        _wload("wq")
        _wload("wk")
        _wload("wv")
        trig = {}
        for nm in ("c", "s"):
            trig[nm] = wp.tile([128, T], F32R, tag=nm, name="trig_" + nm)

        def _late_loads():
            # issued after the first window's x chunks so the DMA queues
            # prioritize what phase 1 needs first
            nc.sync.dma_start(out=trig["c"][:], in_=cd[:])
            nc.sync.dma_start(out=trig["s"][:], in_=sd[:])
            nc.sync.dma_start(out=negi_sb[:], in_=negid[:])
            nc.sync.dma_start(out=step_sb[:], in_=stepd[:])
            for p in range(2):
                nc.sync.dma_start(out=wot_sb[p][:],
                                  in_=wot[p * 128:(p + 1) * 128, :])
            nc.sync.dma_start(out=ones_sb[64:65, :], in_=onesd[0:1, 0:64])
            v_ones_view = v_sb[:].rearrange("p (tt h x) -> p tt h x",
                                            tt=16, h=4)
            nc.sync.dma_start(
                out=v_ones_view[:, :, :, 64:65],
                in_=onesd[:, 0:64].rearrange("p (tt h) -> p tt h",
                                             tt=16)[:, :, :, None])

        # ---------------- phase 1: projections + rope -----------------------
        with tc.tile_pool(name="pp", bufs=1, space="PSUM") as pp:
            for tcx in range(2):
                for half in range(2):
                    c0 = half * 512
                    tw = tcx * 1024 + c0
                    xc = []
                    for eo in range(8):
                        t_ = xcp.tile([128, 512], F32R, tag="xc", name="xc")
                        eng = nc.sync
                        eng.dma_start(
                            out=t_[:],
                            in_=xt[eo * 128:(eo + 1) * 128, tw:tw + 512])
                        xc.append(t_)
                    if tcx == 0 and half == 0:
                        _late_loads()
                    ps = {nm: pp.tile([128, 512], F32, tag="qkps", bufs=4,
                                      name="ps_" + nm)
                          for nm in ("q0", "q1", "k0", "k1")}
                    for eo in range(8):
                        for wnm, dh in (("wq", 0), ("wq", 1),
                                        ("wk", 0), ("wk", 1)):
                            dst = ("q" if wnm == "wq" else "k") + str(dh)
                            nc.tensor.matmul(
                                out=ps[dst][:],
                                lhsT=w_sb[wnm][:, eo * 256 + dh * 128:
                                               eo * 256 + dh * 128 + 128],
                                rhs=xc[eo][:],
                                start=(eo == 0), stop=(eo == 7))
                    for i_, nm in enumerate(("q0", "q1", "k0", "k1")):
                        if i_ % 2 == 0:
                            nc.vector.tensor_copy(
                                qk[nm][:, tw:tw + 512], ps[nm][:])
                        else:
                            nc.scalar.copy(
                                qk[nm][:, tw:tw + 512], ps[nm][:])
                    # V: tt-outer, eo-inner so only 2 psum banks needed
                    for tt_ in range(4):
                        psv = pp.tile([128, 256], F32, tag="vps", bufs=4,
                                      name="psv")
                        for eo in range(8):
                            nc.tensor.matmul(
                                out=psv[:],
                                lhsT=xc[eo][:, tt_ * 128:tt_ * 128 + 128],
                                rhs=w_sb["wv"][:, eo * 256:(eo + 1) * 256],
                                start=(eo == 0), stop=(eo == 7))
                        gt = tw // 128 + tt_
                        dst = v_sb[:, gt * 260:(gt + 1) * 260] \
                            .rearrange("p (h x) -> p h x", h=4)
                        src_ = psv[:].rearrange("p (h x) -> p h x", h=4)
                        if tt_ % 2 == 0:
                            nc.vector.tensor_copy(dst[:, :, 0:64],
                                                  src_[:, :, 0:64])
                        else:
                            nc.scalar.copy(dst[:, :, 0:64],
                                           src_[:, :, 0:64])
                # rope this 1024-window right after its projections
                # (pair-0 tiles first so attention can start early)
                for nm in ("q0", "k0", "q1", "k1"):
                    cs = slice(tcx * 1024, tcx * 1024 + 1024)
                    raw = qk[nm]
                    swp = swpp.tile([128, 1024], F32R, tag="swp")
                    for j in range(4):
                        a, b_ = j * 32, (j ^ 1) * 32
                        nc.sync.dma_start(out=swp[a:a + 32, :],
                                          in_=raw[b_:b_ + 32, cs])
                    nc.vector.tensor_mul(raw[:, cs], raw[:, cs],
                                         trig["c"][:, cs])
                    nc.vector.tensor_mul(swp[:], swp[:], trig["s"][:, cs])
                    nc.vector.tensor_add(raw[:, cs], raw[:, cs], swp[:])

        # ---------------- phase 2 + 3: attention, yT interleaved -----------
        with tc.tile_pool(name="ps_s", bufs=2, space="PSUM") as ps_sp, \
             tc.tile_pool(name="ps_a", bufs=2, space="PSUM") as ps_ap:
            recip_pool = dnp
            ytv = yt.rearrange("(et p) t -> p et t", p=128)

            def yt_chunk(qc, pool, tag):
                # output projection for qt cols [qc*512, qc*512+512)
                for eg in range(2):
                    y_sb = yp.tile([128, 4 * 512], F32, tag="ysb", bufs=2,
                                   name="y_sb")
                    for ei in range(4):
                        et_ = eg * 4 + ei
                        ps_y = pool.tile([128, 512], F32, tag=tag,
                                         name="ps_y")
                        for p in range(2):
                            nc.tensor.matmul(
                                out=ps_y[:],
                                lhsT=wot_sb[p][:, et_ * 128:(et_ + 1) * 128],
                                rhs=at[p][:, qc * 512:(qc + 1) * 512],
                                start=(p == 0), stop=(p == 1))
                        if et_ % 2 == 0:
                            nc.vector.tensor_copy(
                                y_sb[:, ei * 512:(ei + 1) * 512], ps_y[:])
                        else:
                            nc.scalar.copy(
                                y_sb[:, ei * 512:(ei + 1) * 512], ps_y[:])
                    nc.sync.dma_start(
                        out=ytv[:, eg * 4:eg * 4 + 4,
                                qc * 512:(qc + 1) * 512],
                        in_=y_sb[:].rearrange("p (et t) -> p et t", et=4))

            def attention(pair, qtb):
                krot = [qk["k" + str(pair)], qk["k" + str(pair)]]
                qrot = [qk["q" + str(pair)], qk["q" + str(pair)]]
                nkt = 8 * qtb + 8
                q0 = qtb * 1024
                ps_a = [ps_ap.tile([128, 1024], F32, tag="a", name="ps_a")
                        for _ in range(2)]
                exp_tiles = [None] * nkt
                chunk_l = [None] * nkt

                def scores_step(kt):
                    qs_rel = max(0, 128 * kt - q0)
                    chs = _chunks(qs_rel)
                    chunk_l[kt] = chs
                    ps_s = [ps_sp.tile([128, 1024], F32, tag="s",
                                       name="ps_s")
                            for _ in range(2)]
                    diag = kt >= 8 * qtb
                    for h in range(2):
                        for (ca, cb) in chs:
                            is_diag_chunk = diag and ca == qs_rel
                            nc.tensor.matmul(
                                out=ps_s[h][:, ca:cb],
                                lhsT=krot[h][h * 64:h * 64 + 64,
                                             kt * 128:kt * 128 + 128],
                                rhs=qrot[h][h * 64:h * 64 + 64,
                                            q0 + ca:q0 + cb],
                                start=True, stop=not is_diag_chunk,
                                tile_position=(h * 64, 0))
                    if diag:
                        for h in range(2):
                            nc.tensor.matmul(
                                out=ps_s[h][:, qs_rel:qs_rel + 128],
                                lhsT=negi_sb[:],
                                rhs=step_sb[:],
                                start=False, stop=True)
                    et = []
                    for h in range(2):
                        e_ = expp.tile([128, 1024], F32R, tag="e",
                                       name="exp_t")
                        nc.scalar.activation(
                            e_[:, qs_rel:1024], ps_s[h][:, qs_rel:1024],
                            AF.Exp)
                        et.append(e_)
                    exp_tiles[kt] = et

                def attnv_step(kt):
                    # psum stop flags are bank-granular: bank0's last
                    # writer is kt=8*qtb+3, bank1's is nkt-1
                    et = exp_tiles[kt]
                    for h in range(2):
                        slot = kt * 260 + (2 * pair + h) * 65
                        for (ca, cb) in chunk_l[kt]:
                            last = 8 * qtb + 3 if ca < 512 else nkt - 1
                            nc.tensor.matmul(
                                out=ps_a[h][0:65, ca:cb],
                                lhsT=v_sb[:, slot:slot + 65],
                                rhs=et[h][:, ca:cb],
                                start=(kt == 0), stop=(kt == last))
                    exp_tiles[kt] = None

                for step in range(nkt + 1):
                    if step < nkt:
                        scores_step(step)
                    if step > 0:
                        attnv_step(step - 1)

                # normalization: denom rows staged in f32r dh tiles
                # (they feed the K=1 broadcast matmul); recip outputs in
                # plain f32 tiles (only consumed by DVE)
                dh0 = recip_pool.tile([128, 1024], F32R, tag="dh",
                                      name="dh0")
                dh1 = recip_pool.tile([128, 1024], F32R, tag="dh",
                                      name="dh1")
                nc.scalar.copy(dh0[64:65, :], ps_a[0][64:65, :])
                nc.scalar.copy(dh1[64:65, :], ps_a[1][64:65, :])
                ps_b = [ps_sp.tile([128, 1024], F32, tag="s",
                                   name="ps_b") for _ in range(2)]
                dns = (dh0, dh1)
                for h in range(2):
                    for (ca, cb) in ((0, 512), (512, 1024)):
                        nc.tensor.matmul(
                            out=ps_b[h][0:64, ca:cb],
                            lhsT=ones_sb[64:65, :],
                            rhs=dns[h][64:65, ca:cb],
                            start=True, stop=True,
                            tile_position=(64, 0))
                recip = recip_pool.tile([128, 1024], F32, tag="rc")
                scr = recip_pool.tile([128, 1024], F32, tag="scr")
                nc.vector.reciprocal_approx_fast(
                    out=recip[0:64, :], in_=ps_b[0][0:64, :])
                nc.vector.reciprocal_approx_fast(
                    out=scr[0:64, :], in_=ps_b[1][0:64, :])
                nc.vector.tensor_mul(
                    at[pair][0:64, q0:q0 + 1024],
                    ps_a[0][0:64, :], recip[0:64, :])
                a1n = recip_pool.tile([64, 1024], F32R, tag="dh",
                                      name="a1n")
                nc.vector.tensor_mul(
                    a1n[0:64, :], ps_a[1][0:64, :], scr[0:64, :])
                nc.sync.dma_start(
                    out=at[pair][64:128, q0:q0 + 1024], in_=a1n[0:64, :])

            for pair in range(2):
                for qtb in range(2):
                    attention(pair, qtb)

        # ---------------- phase 3: output projection ------------------------
        with tc.tile_pool(name="ps_y", bufs=4, space="PSUM") as ps_yp:
            for qc in range(4):
                yt_chunk(qc, ps_yp, "y")


# ----------------------------------------------------------------- host side
def _prep_core_inputs(x, wq, wk, wv, wo):
    """Build the 8 per-core input dicts (numpy fp32)."""
    # rope trig tables, transposed [freq, pos]
    inv_freq = 1.0 / (ROPE_BASE ** (np.arange(0, HD, 2, dtype=np.float32) / HD))
    pos = np.arange(T, dtype=np.float32)
    freqs = pos[:, None] * inv_freq[None, :]          # [T, 32]
    cosT = np.cos(freqs).T.astype(np.float32)          # [32, T]
    sinT = np.sin(freqs).T.astype(np.float32)
    C = np.tile(cosT, (4, 1)).astype(np.float32)       # [128, T]
    S = np.tile(np.concatenate([-sinT, sinT], axis=0), (2, 1)).astype(np.float32)
    scale = np.float32(1.0 / np.sqrt(HD))              # folded into wq

    import ml_dtypes
    r, c = np.indices((128, 128))
    negid = (np.eye(128) * NEG).astype(ml_dtypes.bfloat16)
    stepd = (c < r).astype(ml_dtypes.bfloat16)

    # per-head de-interleave: rows [even dims, odd dims]
    perm = np.concatenate([np.arange(0, HD, 2), np.arange(1, HD, 2)])

    # batch transposes computed once, shared by the 4 cores of each batch
    xts = [np.ascontiguousarray(x[b_].T) for b_ in range(B)]       # [E, T]
    onesd = np.ones((128, 64), dtype=np.float32)
    in_maps = []
    for core in range(NCORES):
        b_, hg = divmod(core, 4)
        heads = np.arange(4 * hg, 4 * hg + 4)
        rows = np.concatenate([h * HD + perm for h in heads])      # permuted
        rows_plain = np.concatenate([h * HD + np.arange(HD) for h in heads])
        # fancy-index on the transposed views: one contiguous copy each
        wqt_ = wq.T[:, rows] * scale                               # [E, DL]
        wkt_ = np.ascontiguousarray(wk.T[:, rows])
        wvt_ = np.ascontiguousarray(wv.T[:, rows_plain])
        wot_ = np.ascontiguousarray(wo.T[rows_plain, :])           # [DL, E]
        in_maps.append({
            "xt": xts[b_], "wqt": wqt_, "wkt": wkt_, "wvt": wvt_,
            "wot": wot_, "cd": C, "sd": S,
            "negid": negid, "stepd": stepd, "onesd": onesd,
        })
    return in_maps


_NC_CACHE = {}


def _get_module():
    if "nc" not in _NC_CACHE:
        _NC_CACHE["nc"] = build_module()
    return _NC_CACHE["nc"]


def _get_runner(key="nc", builder=None):
    """Build (once) a cached jax.jit shard_map callable over the 8 cores."""
    rkey = "runner_" + key
    if rkey in _NC_CACHE:
        return _NC_CACHE[rkey]
    import jax
    import concourse.mybir as _mb
    from concourse import bass2jax as b2j
    from jax.sharding import Mesh, PartitionSpec
    from jax.experimental.shard_map import shard_map

    if key == "nc":
        nc = _get_module()
    else:
        if key not in _NC_CACHE:
            _NC_CACHE[key] = builder()
        nc = _NC_CACHE[key]
    b2j.install_neuronx_cc_hook()
    partition_name = (nc.partition_id_tensor.name
                      if nc.partition_id_tensor else None)
    in_names, out_names, out_avals, zero_outs = [], [], [], []
    for alloc in nc.m.functions[0].allocations:
        if not isinstance(alloc, _mb.MemoryLocationSet):
            continue
        name = alloc.memorylocations[0].name
        if alloc.kind == "ExternalInput":
            if name != partition_name:
                in_names.append(name)
        elif alloc.kind == "ExternalOutput":
            out_names.append(name)
            shape = tuple(alloc.tensor_shape)
            dtype = _mb.dt.np(alloc.dtype)
            out_avals.append(jax.core.ShapedArray(shape, dtype))
            zero_outs.append(np.zeros(shape, dtype))
    n_params = len(in_names)
    all_names = list(in_names) + list(out_names)
    if partition_name is not None:
        all_names.append(partition_name)

    def _body(*args):
        operands = list(args)
        if partition_name is not None:
            operands.append(b2j.partition_id_tensor())
        outs = b2j._bass_exec_p.bind(
            *operands,
            out_avals=tuple(out_avals),
            in_names=tuple(all_names),
            out_names=tuple(out_names),
            lowering_input_output_aliases=(),
            sim_require_finite=True,
            sim_require_nnan=True,
            nc=nc,
        )
        return tuple(outs)

    devices = jax.devices()[:NCORES]
    mesh = Mesh(np.asarray(devices), ("core",))
    n_outs = len(out_names)
    in_specs = (PartitionSpec("core"),) * (n_params + n_outs)
    out_specs = (PartitionSpec("core"),) * n_outs
    sharded = jax.jit(
        shard_map(_body, mesh=mesh, in_specs=in_specs, out_specs=out_specs,
                  check_rep=False),
        keep_unused=True)
    from jax.sharding import NamedSharding
    _shard = NamedSharding(mesh, PartitionSpec("core"))
    concat_zeros = [
        jax.device_put(
            np.zeros((NCORES * z.shape[0], *z.shape[1:]), z.dtype), _shard)
        for z in zero_outs
    ]
    runner = {
        "sharded": sharded, "in_names": in_names, "out_names": out_names,
        "out_avals": out_avals, "concat_zeros": concat_zeros,
    }
    _NC_CACHE[rkey] = runner
    return runner


_CONST_NAMES = {"cd", "sd", "negid", "stepd", "onesd"}


def _run_spmd_cached(in_maps):
    import jax
    r = _get_runner()
    ckey = "const_dev"
    if ckey not in _NC_CACHE:
        _NC_CACHE[ckey] = {}
    const_dev = _NC_CACHE[ckey]
    concat_in = []
    for nm in r["in_names"]:
        if nm in _CONST_NAMES:
            # identical across cores and across calls: transfer once
            if nm not in const_dev:
                arr = np.concatenate(
                    [np.asarray(in_maps[c][nm]) for c in range(NCORES)],
                    axis=0)
                const_dev[nm] = jax.device_put(arr)
            concat_in.append(const_dev[nm])
        else:
            concat_in.append(np.concatenate(
                [np.asarray(in_maps[c][nm]) for c in range(NCORES)], axis=0))
    out_arrs = r["sharded"](*concat_in, *r["concat_zeros"])
    nm = r["out_names"]
    av = r["out_avals"]
    return [
        {nm[i]: np.asarray(out_arrs[i]).reshape(NCORES, *av[i].shape)[c]
         for i in range(len(nm))}
        for c in range(NCORES)
    ]


def _build_trivial():
    nc = bacc.Bacc("TRN2", target_bir_lowering=False, debug=False,
                   num_devices=NCORES)
    a = nc.dram_tensor("a", [128, 128], F32, kind="ExternalInput").ap()
    b_ = nc.dram_tensor("b", [128, 128], F32, kind="ExternalOutput").ap()
    with tile.TileContext(nc) as tc:
        with tc.tile_pool(name="t", bufs=1) as p:
            t_ = p.tile([128, 128], F32, tag="t")
            nc.sync.dma_start(out=t_[:], in_=a[:])
            nc.sync.dma_start(out=b_[:], in_=t_[:])
    nc.compile()
    return nc


def bench_hw(x, wq, wk, wv, wo, reps=9, n=30):
    """HW per-iteration time from slope: module with body repeated `reps`
    times vs once, both on the same dispatch floor."""
    import time
    import jax
    from jax.sharding import Mesh, NamedSharding, PartitionSpec

    mesh = Mesh(np.asarray(jax.devices()[:NCORES]), ("core",))
    shard = NamedSharding(mesh, PartitionSpec("core"))

    def timed(runner, concat_in):
        f = runner["sharded"]
        zs = runner["concat_zeros"]
        out = f(*concat_in, *zs)
        out[0].block_until_ready()
        ts = []
        for _ in range(n):
            t0 = time.perf_counter()
            o = f(*concat_in, *zs)
            o[0].block_until_ready()
            ts.append(time.perf_counter() - t0)
        ts.sort()
        return ts[0], ts[len(ts) // 2]

    in_maps = _prep_core_inputs(x, wq, wk, wv, wo)

    def concat(runner):
        return [
            jax.device_put(np.concatenate(
                [np.asarray(in_maps[c][nm]) for c in range(NCORES)], axis=0),
                shard)
            for nm in runner["in_names"]
        ]

    r1 = _get_runner()
    t1_min, t1_med = timed(r1, concat(r1))
    rR = _get_runner(f"nc_r{reps}", lambda: build_module(reps=reps))
    tR_min, tR_med = timed(rR, concat(rR))
    per_min = (tR_min - t1_min) / (reps - 1)
    per_med = (tR_med - t1_med) / (reps - 1)
    print(f"  x1: min {t1_min*1e3:.3f} med {t1_med*1e3:.3f} ms ; "
          f"x{reps}: min {tR_min*1e3:.3f} med {tR_med*1e3:.3f} ms")
    print(f"HW exec time: {per_min*1e9:.0f} ns (min)  {per_med*1e9:.0f} ns (med)")
    return per_min


def kernel(x, wq, wk, wv, wo, _trace=False, _trace_kwargs=None):
    x = np.asarray(x, dtype=np.float32)
    wq = np.asarray(wq, dtype=np.float32)
    wk = np.asarray(wk, dtype=np.float32)
    wv = np.asarray(wv, dtype=np.float32)
    wo = np.asarray(wo, dtype=np.float32)

    in_maps = _prep_core_inputs(x, wq, wk, wv, wo)
    try:
        results = _run_spmd_cached(in_maps)
    except Exception:
        nc = _get_module()
        results = run_bass_kernel_spmd(
            nc, in_maps, core_ids=list(range(NCORES))).results
    out = np.empty((B, T, E), dtype=np.float32)
    for b_ in range(B):
        acc = np.zeros((E, T), dtype=np.float32)
        for g in range(4):
            acc += results[4 * b_ + g]["yt"]
        out[b_] = acc.T
    return out


if __name__ == "__main__":
    nc = _get_module()
    print("module built ok")



# revision 39
# speedup vs baseline: 1.2996x; 1.2996x over previous
"""Multi-head self-attention (16 heads, hd=64, RoPE, causal) on 8 trn2 cores.

Sharding: DP(batch=2) x TP(head-groups=4). Core c handles batch c//4, heads
[4*(c%4), 4*(c%4)+4). Each core computes a row-parallel partial output
yT_partial [1024, 2048]; host sums the 4 partials per batch and transposes.
No device-device communication.

Device kernel (v2, software-pipelined):
  - bf16 x / wqkv / wo / q / k / v / trig / exp-weights / normalized
    attention; fp32 PSUM accumulation and fp32 output partials.
  - transposed layout throughout: xT [e,t], qT/kT [128, t] per head-pair
    (per-head rows de-interleaved [evens|odds] so the RoPE partner swap is
    row^32), scoresT [kt, q] per head, attnT via v_aug ones-column trick.
  - RoPE partner swap computed on PE with a 0/1 permutation matmul
    (no partition-swap DMAs).
  - one exp activation per kt step covering both heads of a pair (3D AP over
    a [128, 1024] PSUM tile).
  - attention emitted as q-windows of 512 cols; a filler FIFO interleaves
    projection / output-projection matmuls between attention steps so the
    PE never idles (the cost model halves PE speed for 3us after any idle).
  - warmup matmuls on a zeroed tile bridge the initial DMA wait.
"""

import sys

for _p in ("/opt/trn_rl_repo",):
    if _p not in sys.path:
        sys.path.insert(0, _p)

from collections import deque
from contextlib import ExitStack

import numpy as np

import concourse.bass as bass
import concourse.mybir as mybir
import concourse.tile as tile
from concourse import bacc
from concourse.bass_utils import run_bass_kernel_spmd

F32 = mybir.dt.float32
F32R = mybir.dt.float32r
BF16 = mybir.dt.bfloat16
AF = mybir.ActivationFunctionType

B, T, E = 2, 2048, 1024
NH, HD = 16, 64
NHL = 4          # heads per core
DL = NHL * HD    # 256 local head dims
NCORES = 8
NEG = -1e9
ROPE_BASE = 10000.0

QW = 512         # attention q-window
NWIN = T // QW   # 4 windows
NBLK = 4         # projection t-blocks of 512
N_WARM = 52      # warmup matmuls (N=256) bridging the initial DMA wait
N_BRIDGE = 60    # tail-bridge matmuls through the last norm/DMA latency


# ----------------------------------------------------------------- device IR
def build_module(reps=1):
    nc = bacc.Bacc("TRN2", target_bir_lowering=False, debug=False,
                   num_devices=NCORES)

    xt = nc.dram_tensor("xt", [E, T], BF16, kind="ExternalInput").ap()
    wqkv = nc.dram_tensor("wqkv", [E, 3 * DL], BF16, kind="ExternalInput").ap()
    wot = nc.dram_tensor("wot", [DL, E], BF16, kind="ExternalInput").ap()
    trig = nc.dram_tensor("trig", [2, 128, T], BF16, kind="ExternalInput").ap()
    consts = nc.dram_tensor("consts", [128, 384], BF16,
                            kind="ExternalInput").ap()
    yt = nc.dram_tensor("yt", [E, T], F32, kind="ExternalOutput").ap()

    with tile.TileContext(nc) as tc:
        for _ in range(reps):
            _body(tc, xt, wqkv, wot, trig, consts, yt)
    nc.compile()
    return nc


def _body(tc, xt, wqkv, wot, trig, consts, yt):
    nc = tc.nc

    with ExitStack() as ctx:
        po = ctx.enter_context(tc.tile_pool(name="po", bufs=1))
        xcp = ctx.enter_context(tc.tile_pool(name="xcp", bufs=16))
        rp = ctx.enter_context(tc.tile_pool(name="rp", bufs=3))
        ep = ctx.enter_context(tc.tile_pool(name="ep", bufs=4))
        dp = ctx.enter_context(tc.tile_pool(name="dp", bufs=3))
        yp = ctx.enter_context(tc.tile_pool(name="yp", bufs=1))
        pjp = ctx.enter_context(tc.tile_pool(name="pjp", bufs=2, space="PSUM"))
        ssp = ctx.enter_context(tc.tile_pool(name="ssp", bufs=2, space="PSUM"))
        sap = ctx.enter_context(tc.tile_pool(name="sap", bufs=2, space="PSUM"))

        # ---------------- persistent tiles --------------------------------
        # qk[0]=q pair0, qk[1]=q pair1, qk[2]=k pair0, qk[3]=k pair1
        qk = [po.tile([128, T], BF16, tag=f"qk{i}", name=f"qk{i}")
              for i in range(4)]
        v_sb = po.tile([128, 16 * 260], BF16, tag="v", name="v_sb")
        w_sb = po.tile([128, 8 * 768], BF16, tag="w", name="w_sb")
        wot_sb = [po.tile([128, E], BF16, tag=f"wot{p}", name=f"wot{p}")
                  for p in range(2)]
        trigc = po.tile([128, T], BF16, tag="tc", name="trigc")
        trigs = po.tile([128, T], BF16, tag="tsn", name="trigs")
        cst = po.tile([128, 384], BF16, tag="cst", name="cst")
        negi, stepm, permm = cst[:, 0:128], cst[:, 128:256], cst[:, 256:384]
        at = [po.tile([128, T], BF16, tag=f"at{p}", name=f"at{p}")
              for p in range(2)]
        warm = po.tile([128, 384], BF16, tag="warm", name="warm")

        ones_sb = po.tile([65, 64], F32, tag="ones", name="ones_sb")

        # ---------------- init: memsets + DMAs -----------------------------
        nc.gpsimd.memset(warm[:], 0.0)
        nc.gpsimd.memset(ones_sb[64:65, 0:64], 1.0)
        v_ones = v_sb[:].rearrange("p (kt h x) -> p kt h x", kt=16, h=4)
        nc.gpsimd.memset(v_ones[:, :, :, 64:65], 1.0)

        w_v = w_sb[:].rearrange("p (eo d) -> p eo d", eo=8)
        wqkv_v = wqkv.rearrange("(eo p) d -> p eo d", p=128)
        # issue order tuned so each transfer lands just before first use
        nc.sync.dma_start(out=w_v[:, :, 0:256], in_=wqkv_v[:, :, 0:256])

        xc = {}

        def load_x(b):
            for eo in range(8):
                t_ = xcp.tile([128, 512], BF16, tag="xc", name="xc")
                nc.sync.dma_start(
                    out=t_[:],
                    in_=xt[eo * 128:(eo + 1) * 128,
                           b * 512:(b + 1) * 512])
                xc[(b, eo)] = t_

        load_x(0)
        nc.sync.dma_start(out=w_v[:, :, 256:512], in_=wqkv_v[:, :, 256:512])
        nc.sync.dma_start(out=w_v[:, :, 512:768], in_=wqkv_v[:, :, 512:768])
        nc.sync.dma_start(out=trigc[:], in_=trig[0])
        nc.sync.dma_start(out=trigs[:], in_=trig[1])
        load_x(1)
        nc.sync.dma_start(out=cst[:], in_=consts[:])
        load_x(2)
        load_x(3)
        for p in range(2):
            nc.sync.dma_start(out=wot_sb[p][:],
                              in_=wot[p * 128:(p + 1) * 128, :])

        # activation-table load lands during the DMA wait
        nc.scalar.activation(warm[0:1, 256:257], warm[0:1, 0:1], AF.Exp)

        # warmup: keep PE busy (and ramping) until the first x chunks land
        warm_ps = pjp.tile([128, 256], F32, tag="pj", name="warm_ps")
        for i in range(N_WARM):
            nc.tensor.matmul(out=warm_ps[:], lhsT=warm[:, 0:128],
                             rhs=warm[:, 128:384],
                             start=(i == 0), stop=(i == N_WARM - 1))

        # ---------------- projection + rope emission helpers ---------------
        # nm: 0=q0, 1=q1, 2=k0, 3=k1 ; block b covers t cols [512b, 512b+512)
        def qk_mms(nm, b):
            """8 accumulating MMs + psum->bf16 copy; returns raw/psum tiles."""
            wcol = (nm % 2) * 128 if nm < 2 else 256 + (nm % 2) * 128
            ps = pjp.tile([128, 512], F32, tag="pj", name="pjqk")
            for eo in range(8):
                nc.tensor.matmul(
                    out=ps[:],
                    lhsT=w_sb[:, eo * 768 + wcol: eo * 768 + wcol + 128],
                    rhs=xc[(b, eo)][:],
                    start=(eo == 0), stop=(eo == 7))
            raw = rp.tile([128, 512], BF16, tag="raw", name="raw")
            if b == 0:
                nc.scalar.copy(raw[:], ps[:])   # ACT is idle pre-attention
            else:
                nc.vector.tensor_copy(raw[:], ps[:])
            return raw

        def qk_rope(nm, b, raw):
            """perm matmul + cos/sin muls + add into qk[nm] block cols."""
            cs = slice(b * 512, b * 512 + 512)
            swp = pjp.tile([128, 512], F32, tag="pj", name="pjswp")
            nc.tensor.matmul(out=swp[:], lhsT=permm, rhs=raw[:],
                             start=True, stop=True)
            nc.gpsimd.tensor_mul(qk[nm][:, cs], raw[:], trigc[:, cs])
            tmp = rp.tile([128, 512], BF16, tag="tmp", name="tmp")
            nc.vector.tensor_mul(tmp[:], swp[:], trigs[:, cs])
            nc.gpsimd.tensor_add(qk[nm][:, cs], qk[nm][:, cs], tmp[:])

        def v_grp(b, tt):
            """one 128-t-row V projection group; kt block = 4b+tt."""
            ps = pjp.tile([128, 256], F32, tag="pj", name="pjv")
            for eo in range(8):
                nc.tensor.matmul(
                    out=ps[:],
                    lhsT=xc[(b, eo)][:, tt * 128:tt * 128 + 128],
                    rhs=w_v[:, eo, 512:768],
                    start=(eo == 0), stop=(eo == 7))
            kt = 4 * b + tt
            dst = v_sb[:, kt * 260:(kt + 1) * 260] \
                .rearrange("p (h x) -> p h x", h=4)
            if b == 0:
                nc.scalar.copy(dst[:, :, 0:64],
                               ps[:].rearrange("p (h x) -> p h x", h=4))
            else:
                nc.vector.tensor_copy(dst[:, :, 0:64],
                                      ps[:].rearrange("p (h x) -> p h x", h=4))

        # ---------------- filler FIFO --------------------------------------
        fifo = deque()
        emitted = set()
        # rough PE-time of each item kind, for the debt-based pump
        COSTS = {"qkA": 1700.0, "qkB": 260.0, "v": 900.0, "yt": 480.0}
        debt = [0.0]

        def enqueue_block(b, b0_order=False):
            raws = {}
            if b0_order:
                # DMA arrival order at startup: A's first, then v/B
                # interleaved (psum-ring WARs hide behind alternation)
                order = [("A", 0), ("A", 2), ("A", 1), ("A", 3),
                         ("v", 0), ("B", 0), ("v", 1), ("B", 2),
                         ("v", 2), ("B", 1), ("v", 3), ("B", 3)]
            else:
                order = [("A", 0), ("A", 2), ("B", 0), ("A", 1), ("B", 2),
                         ("A", 3), ("B", 1), ("B", 3),
                         ("v", 0), ("v", 1), ("v", 2), ("v", 3)]
            for kind, x in order:
                if kind == "A":
                    fifo.append((("qkA", x, b),
                                 lambda nm=x, b=b: raws.__setitem__(
                                     nm, qk_mms(nm, b))))
                elif kind == "B":
                    fifo.append((("qkB", x, b),
                                 lambda nm=x, b=b: qk_rope(nm, b,
                                                           raws.pop(nm))))
                else:
                    fifo.append((("v", b, x),
                                 lambda b=b, tt=x: v_grp(b, tt)))

        def emit_next():
            tag, fn = fifo.popleft()
            fn()
            emitted.add(tag)

        reserve = [0]

        def pump_ns(ns):
            debt[0] += ns
            while len(fifo) > reserve[0] and debt[0] >= COSTS[fifo[0][0][0]]:
                k = fifo[0][0][0]
                emit_next()
                debt[0] -= COSTS[k]

        def force(tag):
            if tag in emitted:
                return
            while fifo:
                t, _ = fifo[0]
                emit_next()
                if t == tag:
                    debt[0] = 0.0
                    return
            raise AssertionError(f"force: {tag} never enqueued")

        # ---------------- attention ----------------------------------------
        def att_call(P, W):
            """attention for pair P, q cols [512W, 512W+512)."""
            nkt = 4 * W + 4
            qcols = slice(W * 512, W * 512 + 512)
            # rope of q[P] block W and k[P] blocks <= W must be emitted
            force(("qkB", P, W))
            for bb in range(W + 1):
                force(("qkB", 2 + P, bb))

            ps_a = [sap.tile([128, 512], F32, tag="a", name="ps_a")
                    for _ in range(2)]
            exps = [None] * nkt

            def scores_step(kt):
                qs = max(0, 128 * kt - 512 * W)
                diag = kt >= 4 * W
                ss = ssp.tile([128, 1024], F32, tag="s", name="ss")
                for h in range(2):
                    nc.tensor.matmul(
                        out=ss[:, h * 512 + qs: h * 512 + 512],
                        lhsT=qk[2 + P][h * 64:h * 64 + 64,
                                       kt * 128:kt * 128 + 128],
                        rhs=qk[P][h * 64:h * 64 + 64, W * 512 + qs:
                                  W * 512 + 512],
                        start=True, stop=not diag,
                        tile_position=(h * 64, 0))
                if diag:
                    for h in range(2):
                        nc.tensor.matmul(
                            out=ss[:, h * 512 + qs: h * 512 + qs + 128],
                            lhsT=negi, rhs=stepm,
                            start=False, stop=True)
                e = ep.tile([128, 1024], BF16, tag="e", name="exp_t")
                e3 = e[:].rearrange("p (h c) -> p h c", h=2)[:, :, qs:512]
                s3 = ss[:].rearrange("p (h c) -> p h c", h=2)[:, :, qs:512]
                nc.scalar.activation(e3, s3, AF.Exp)
                exps[kt] = (e, qs)

            def attnv_step(kt):
                e, qs = exps[kt]
                for h in range(2):
                    slot = kt * 260 + (2 * P + h) * 65
                    nc.tensor.matmul(
                        out=ps_a[h][0:65, qs:512],
                        lhsT=v_sb[:, slot:slot + 65],
                        rhs=e[:, h * 512 + qs: h * 512 + 512],
                        start=(kt == 0), stop=(kt == nkt - 1))
                exps[kt] = None

            for step in range(nkt + 1):
                if step < nkt:
                    # pre-force v blocks one block ahead of the kt cursor
                    vb = min(step // 4 + 1, W)
                    for bb in range(vb + 1):
                        for tt in range(4):
                            if (("v", bb, tt)) not in emitted:
                                force(("v", bb, tt))
                    scores_step(step)
                if step > 0:
                    attnv_step(step - 1)
                # ACT-vs-PE imbalance this step, paid to the filler pump
                qs = max(0, 128 * min(step, nkt - 1) - 512 * W)
                cols = 512 - qs
                gap = (2 * cols * 0.833 + 500.0) - (4 * cols * 0.4167 + 107.0)
                pump_ns(max(200.0, gap))

            # ---------------- normalization -------------------------------
            # denominators: pbcast psum row 64 -> [64,512], recip, then mul.
            # h1 first: its a1n staging DMA is the longest pole into yt.
            last_call = (P == 1 and W == NWIN - 1)
            if last_call:
                # dependency-free bridge: keep PE busy (and un-throttled)
                # through the norm + a1n-DMA latency before yt starts
                bridge_ps = pjp.tile([128, 256], F32, tag="pj",
                                     name="bridge_ps")
                for i in range(N_BRIDGE):
                    nc.tensor.matmul(out=bridge_ps[:], lhsT=warm[:, 0:128],
                                     rhs=warm[:, 128:384],
                                     start=(i == 0), stop=(i == N_BRIDGE - 1))
            pump_ns(600.0)
            # stage denom rows to SBUF (ACT), then K=1 broadcast matmul
            dh = [dp.tile([65, 512], F32R, tag="dh", name="dh")
                  for _ in range(2)]
            nc.scalar.copy(dh[1][64:65, :], ps_a[1][64:65, :])
            nc.scalar.copy(dh[0][64:65, :], ps_a[0][64:65, :])
            ps_b = [None, None]
            for h in (1, 0):
                ps_b[h] = pjp.tile([128, 512], F32, tag="pj", name="ps_b")
                nc.tensor.matmul(out=ps_b[h][0:64, :],
                                 lhsT=ones_sb[64:65, 0:64].bitcast(F32R),
                                 rhs=dh[h][64:65, :],
                                 start=True, stop=True,
                                 tile_position=(64, 0))
            rc = [dp.tile([64, 512], F32, tag="rc", name="rc")
                  for _ in range(2)]
            nc.vector.reciprocal_approx_fast(out=rc[1][0:64, :],
                                             in_=ps_b[1][0:64, :])
            pump_ns(600.0)
            a1n = dp.tile([64, 512], BF16, tag="a1n", name="a1n")
            nc.vector.tensor_mul(a1n[0:64, :], ps_a[1][0:64, :],
                                 rc[1][0:64, :])
            nc.sync.dma_start(out=at[P][64:128, qcols], in_=a1n[0:64, :])
            nc.vector.reciprocal_approx_fast(out=rc[0][0:64, :],
                                             in_=ps_b[0][0:64, :])
            pump_ns(600.0)
            nc.vector.tensor_mul(at[P][0:64, qcols], ps_a[0][0:64, :],
                                 rc[0][0:64, :])

        # ---------------- output projection --------------------------------
        ytv = yt.rearrange("(et p) t -> p et t", p=128)

        def enqueue_yt(W):
            qcols = slice(W * 512, W * 512 + 512)
            y_sb = yp.tile([128, 8 * 512], F32, tag="ysb", name="y_sb")

            nst = 2 if W == NWIN - 1 else 4   # store granularity (ets)

            def yt_grp(et):
                ps_y = pjp.tile([128, 512], F32, tag="pj", name="ps_y")
                for p in range(2):
                    nc.tensor.matmul(
                        out=ps_y[:],
                        lhsT=wot_sb[p][:, et * 128:(et + 1) * 128],
                        rhs=at[p][:, qcols],
                        start=(p == 0), stop=(p == 1))
                ydst = y_sb[:, et * 512:(et + 1) * 512]
                if W == NWIN - 1 and et % 2 == 0:
                    # exp stream is finished: ACT takes half the tail copies
                    nc.scalar.copy(ydst, ps_y[:])
                else:
                    nc.vector.tensor_copy(ydst, ps_y[:])
                if et % nst == nst - 1:
                    eg = et // nst
                    nc.sync.dma_start(
                        out=ytv[:, eg * nst:(eg + 1) * nst, qcols],
                        in_=y_sb[:, eg * nst * 512:(eg + 1) * nst * 512]
                        .rearrange("p (et t) -> p et t", et=nst))

            for et in range(8):
                fifo.append((("yt", W, et), lambda et=et: yt_grp(et)))

        # ---------------- master schedule -----------------------------------
        # block 0 emitted straight; blocks 1..3 via the FIFO
        enqueue_block(0, b0_order=True)
        while fifo:
            emit_next()
        for b in range(1, NBLK):
            enqueue_block(b)

        RESV = {(0, 2): 8, (1, 2): 8, (0, 3): 8, (1, 3): 0}
        for W in range(NWIN):
            for P in range(2):
                reserve[0] = RESV.get((P, W), 0)
                att_call(P, W)
            enqueue_yt(W)
        while fifo:
            emit_next()


# ----------------------------------------------------------------- host side
def _prep_core_inputs(x, wq, wk, wv, wo):
    """Build the 8 per-core input dicts."""
    import ml_dtypes
    bf = ml_dtypes.bfloat16

    inv_freq = 1.0 / (ROPE_BASE ** (np.arange(0, HD, 2, dtype=np.float32) / HD))
    pos = np.arange(T, dtype=np.float32)
    freqs = pos[:, None] * inv_freq[None, :]          # [T, 32]
    cosT = np.cos(freqs).T.astype(np.float32)          # [32, T]
    sinT = np.sin(freqs).T.astype(np.float32)
    C = np.tile(cosT, (4, 1))                          # [128, T]
    S = np.tile(np.concatenate([-sinT, sinT], axis=0), (2, 1))
    trig = np.stack([C, S]).astype(bf)                 # [2, 128, T]
    scale = np.float32(1.0 / np.sqrt(HD))              # folded into wq

    negid = (np.eye(128, dtype=np.float32) * NEG)
    r, c = np.indices((128, 128))
    stepd = (c < r).astype(np.float32)
    permM = (r == (c ^ 32)).astype(np.float32)         # lhsT[p,r]=1 iff p=r^32
    consts = np.concatenate([negid, stepd, permM], axis=1).astype(bf)

    perm64 = np.concatenate([np.arange(0, HD, 2), np.arange(1, HD, 2)])

    xts = [np.ascontiguousarray(x[b_].T).astype(bf) for b_ in range(B)]
    in_maps = []
    for core in range(NCORES):
        b_, hg = divmod(core, 4)
        heads = np.arange(4 * hg, 4 * hg + 4)
        qk_rows = np.concatenate([h * HD + perm64 for h in heads])
        v_rows = np.concatenate([h * HD + np.arange(HD) for h in heads])
        wq_t = wq.T[:, qk_rows] * scale                # [E, 256]
        wk_t = wk.T[:, qk_rows]
        wv_t = wv.T[:, v_rows]
        wqkv = np.concatenate([wq_t, wk_t, wv_t], axis=1).astype(bf)
        wot_ = np.ascontiguousarray(wo.T[v_rows, :]).astype(bf)
        in_maps.append({
            "xt": xts[b_], "wqkv": wqkv, "wot": wot_,
            "trig": trig, "consts": consts,
        })
    return in_maps


_NC_CACHE = {}


def _get_module():
    if "nc" not in _NC_CACHE:
        _NC_CACHE["nc"] = build_module()
    return _NC_CACHE["nc"]


def _get_runner(key="nc", builder=None):
    """Build (once) a cached jax.jit shard_map callable over the 8 cores."""
    rkey = "runner_" + key
    if rkey in _NC_CACHE:
        return _NC_CACHE[rkey]
    import jax
    import concourse.mybir as _mb
    from concourse import bass2jax as b2j
    from jax.sharding import Mesh, PartitionSpec
    from jax.experimental.shard_map import shard_map

    if key == "nc":
        nc = _get_module()
    else:
        if key not in _NC_CACHE:
            _NC_CACHE[key] = builder()
        nc = _NC_CACHE[key]
    b2j.install_neuronx_cc_hook()
    partition_name = (nc.partition_id_tensor.name
                      if nc.partition_id_tensor else None)
    in_names, out_names, out_avals, zero_outs = [], [], [], []
    for alloc in nc.m.functions[0].allocations:
        if not isinstance(alloc, _mb.MemoryLocationSet):
            continue
        name = alloc.memorylocations[0].name
        if alloc.kind == "ExternalInput":
            if name != partition_name:
                in_names.append(name)
        elif alloc.kind == "ExternalOutput":
            out_names.append(name)
            shape = tuple(alloc.tensor_shape)
            dtype = _mb.dt.np(alloc.dtype)
            out_avals.append(jax.core.ShapedArray(shape, dtype))
            zero_outs.append(np.zeros(shape, dtype))
    n_params = len(in_names)
    all_names = list(in_names) + list(out_names)
    if partition_name is not None:
        all_names.append(partition_name)

    def _body_fn(*args):
        operands = list(args)
        if partition_name is not None:
            operands.append(b2j.partition_id_tensor())
        outs = b2j._bass_exec_p.bind(
            *operands,
            out_avals=tuple(out_avals),
            in_names=tuple(all_names),
            out_names=tuple(out_names),
            lowering_input_output_aliases=(),
            sim_require_finite=True,
            sim_require_nnan=True,
            nc=nc,
        )
        return tuple(outs)

    devices = jax.devices()[:NCORES]
    mesh = Mesh(np.asarray(devices), ("core",))
    n_outs = len(out_names)
    in_specs = (PartitionSpec("core"),) * (n_params + n_outs)
    out_specs = (PartitionSpec("core"),) * n_outs
    sharded = jax.jit(
        shard_map(_body_fn, mesh=mesh, in_specs=in_specs,
                  out_specs=out_specs, check_rep=False),
        keep_unused=True)
    from jax.sharding import NamedSharding
    _shard = NamedSharding(mesh, PartitionSpec("core"))
    concat_zeros = [
        jax.device_put(
            np.zeros((NCORES * z.shape[0], *z.shape[1:]), z.dtype), _shard)
        for z in zero_outs
    ]
    runner = {
        "sharded": sharded, "in_names": in_names, "out_names": out_names,
        "out_avals": out_avals, "concat_zeros": concat_zeros,
    }
    _NC_CACHE[rkey] = runner
    return runner


_CONST_NAMES = {"trig", "consts"}


def _run_spmd_cached(in_maps):
    import jax
    r = _get_runner()
    ckey = "const_dev"
    if ckey not in _NC_CACHE:
        _NC_CACHE[ckey] = {}
    const_dev = _NC_CACHE[ckey]
    concat_in = []
    for nm in r["in_names"]:
        if nm in _CONST_NAMES:
            if nm not in const_dev:
                arr = np.concatenate(
                    [np.asarray(in_maps[c][nm]) for c in range(NCORES)],
                    axis=0)
                const_dev[nm] = jax.device_put(arr)
            concat_in.append(const_dev[nm])
        else:
            concat_in.append(np.concatenate(
                [np.asarray(in_maps[c][nm]) for c in range(NCORES)], axis=0))
    out_arrs = r["sharded"](*concat_in, *r["concat_zeros"])
    nm = r["out_names"]
    av = r["out_avals"]
    return [
        {nm[i]: np.asarray(out_arrs[i]).reshape(NCORES, *av[i].shape)[c]
         for i in range(len(nm))}
        for c in range(NCORES)
    ]


def kernel(x, wq, wk, wv, wo, _trace=False, _trace_kwargs=None):
    x = np.asarray(x, dtype=np.float32)
    wq = np.asarray(wq, dtype=np.float32)
    wk = np.asarray(wk, dtype=np.float32)
    wv = np.asarray(wv, dtype=np.float32)
    wo = np.asarray(wo, dtype=np.float32)

    in_maps = _prep_core_inputs(x, wq, wk, wv, wo)
    try:
        results = _run_spmd_cached(in_maps)
    except Exception:
        nc = _get_module()
        results = run_bass_kernel_spmd(
            nc, in_maps, core_ids=list(range(NCORES))).results
    out = np.empty((B, T, E), dtype=np.float32)
    for b_ in range(B):
        acc = np.zeros((E, T), dtype=np.float32)
        for g in range(4):
            acc += results[4 * b_ + g]["yt"]
        out[b_] = acc.T
    return out


if __name__ == "__main__":
    nc = _get_module()
    print("module built ok")
